# revision 25
# baseline (speedup 1.0000x reference)
"""Multi-head attention (B=2, S=2048, D=2048, H=16, hd=128) on 8 TRN2 NeuronCores.

Sharding: data-parallel over batch (2) x tensor-parallel over head groups (4).
Core c handles batch c//4 and heads [4*(c%4), 4*(c%4)+4). Each core computes
q/k/v projections for its 512 features, RoPE, full attention over S for its 4
heads, and a partial output projection y_partial = attn_local @ wo[:, cols].T.
Host sums the 4 partials per batch (no on-chip collectives).

All matmuls run in f16 with fp32 PSUM accumulation. The 1/sqrt(hd) score
scale is folded into wq host-side. RoPE pairs are split even/odd across the
partition dim by permuting wq/wk rows host-side, so RoPE is elementwise DVE
work against stacked [cos;cos] / [sin;sin] tables. Scores are computed
transposed ([k, q]) so softmax(exp)@V needs no on-chip transposes; the
softmax denominator is accumulated on DVE, summed across partitions with a
ones-matmul on the PE, and divided out after the PV matmul. PSUM->SBUF
evictions run on DVE so ScalarE does nothing but the softmax exps.

Emission order is a software pipeline that keeps TensorE dense: k proj
(chunk 0 dc-outer against quarter-granular DMAs so the first matmul gates
on ~1MB of weights+x instead of 4MB), q proj, v proj (its first two row
tiles are emitted inside the phase-1 PSUM pool so the pool-reuse barrier
hides under real matmuls; exp hides under the v GEMM), then steady-state
[pv(b) | scores(b+2) | projection(finished q-chunk)]. The denominator
ones-matmul for block b+1 is emitted mid pv-block b with its reciprocal
right behind it, so the attn multiply never waits on PSUM. y partials are
written fp16 (host accumulates in fp32) to halve output DMA; the final
row tile is DMA'd in quarter slices so only ~128KB drains after the last
matmul.
"""

import numpy as np

B = 2
S = 2048
D = 2048
H = 16
HD = 128
P = 128
N_CORES = 8
H_LOC = 4          # heads per core
F = H_LOC * HD     # local features = 512
NCH = 4            # n-chunks of 512 over S
CH = S // NCH      # 512
DCH = D // P       # 16 contraction chunks
NT = S // P        # 16 row tiles
NQ = 8             # dc slices for startup DMA granularity
DQ = DCH // NQ     # 2 dc per slice

_F16 = np.float16


def _build_program():
    import concourse.bass_isa as bass_isa
    import concourse.mybir as mybir
    import concourse.tile as tile
    from concourse import bacc

    dt = mybir.dt
    nc = bacc.Bacc("TRN2", target_bir_lowering=False, debug=False,
                   num_devices=N_CORES)

    # partition-major layouts so every DMA reads >=2KB contiguous per line
    xTc = nc.dram_tensor("xTc", [NCH, P, DCH, CH], dt.float16,
                         kind="ExternalInput").ap()
    wqT = nc.dram_tensor("wqT", [P, DCH, F], dt.float16,
                         kind="ExternalInput").ap()
    wkT = nc.dram_tensor("wkT", [P, DCH, F], dt.float16,
                         kind="ExternalInput").ap()
    wvT = nc.dram_tensor("wvT", [P, DCH, F], dt.float16,
                         kind="ExternalInput").ap()
    woT = nc.dram_tensor("woT", [P, H_LOC, D], dt.float16,
                         kind="ExternalInput").ap()
    # stacked RoPE tables: [cos;cos] and [sin;sin]
    ct = nc.dram_tensor("ct", [P, S], dt.float16, kind="ExternalInput").ap()
    st = nc.dram_tensor("st", [P, S], dt.float16, kind="ExternalInput").ap()
    # all-ones stationary operand: softmax denominators as PE row-sums
    ones = nc.dram_tensor("ones", [P, P], dt.float16,
                          kind="ExternalInput").ap()
    y = nc.dram_tensor("y", [S, D], dt.float16, kind="ExternalOutput").ap()

    y3 = y.rearrange("(o p) n -> p o n", p=P)        # [128, 16, 2048]

    NB = NCH * H_LOC  # 16 attention blocks, b = qc*4 + h

    with tile.TileContext(nc) as tc:
        with (
            tc.tile_pool(name="persist", bufs=1) as pp,
            tc.tile_pool(name="xcp", bufs=2) as xcp,
        ):
            qTp = pp.tile([P, H_LOC, S], dt.float16, tag="qTp")
            kTp = pp.tile([P, H_LOC, S], dt.float16, tag="kTp")
            v_sb = pp.tile([P, NT, F], dt.float16, tag="v")
            wv_sb = pp.tile([P, DCH, F], dt.float16, tag="wv")
            wo_sb = pp.tile([P, H_LOC, D], dt.float16, tag="wo")
            # dedicated buffer: keeps the v-projection x chunk out of the
            # xcp recycle ring so its DMA isn't gated on q-proj progress
            vxc0 = pp.tile([P, DCH, CH], dt.float16, tag="vxc0")
            ones_sb = pp.tile([P, P], dt.float16, tag="ones")

            # ---- phase 1: k and q projections + RoPE ---------------------
            with (
                tc.tile_pool(name="wp", bufs=1) as wp,
                tc.tile_pool(name="t2p", bufs=3) as t2p,
                tc.tile_pool(name="psg", bufs=1, space="PSUM") as psg,
            ):
                wk_q = [wp.tile([P, DQ, F], dt.float16, tag=f"wk_q{g}",
                                name=f"wk_q{g}") for g in range(NQ)]
                wq_q = [wp.tile([P, DQ, F], dt.float16, tag=f"wq_q{g}",
                                name=f"wq_q{g}") for g in range(NQ)]
                xc0_q = [wp.tile([P, DQ, CH], dt.float16, tag=f"xc0_q{g}",
                                 name=f"xc0_q{g}") for g in range(NQ)]
                ct_sb = wp.tile([P, S], dt.float16, tag="ct")
                st_sb = wp.tile([P, S], dt.float16, tag="st")

                # DMA issue order = need order: slice-granular so the
                # very first matmul gates on ~0.5MB, not 4MB. xc1 is issued
                # early in the Sync queue so its completion signal isn't
                # stuck behind the remaining trigger backlog when chunk 1
                # starts.
                xc1 = xcp.tile([P, DCH, CH], dt.float16, tag="xc")
                for g in range(NQ):
                    nc.sync.dma_start(wk_q[g][:], wkT[:, g * DQ:(g + 1) * DQ, :])
                    nc.sync.dma_start(xc0_q[g][:],
                                      xTc[0][:, g * DQ:(g + 1) * DQ, :])
                    if g == 1:
                        nc.sync.dma_start(xc1[:], xTc[1])
                nc.sync.dma_start(ct_sb[:], ct[:])
                nc.sync.dma_start(st_sb[:], st[:])
                for g in range(NQ):
                    nc.sync.dma_start(wq_q[g][:], wqT[:, g * DQ:(g + 1) * DQ, :])

                def rope(ps, h, nsl, outT):
                    # RoPE: partitions 0:64 = even pairs e, 64:128 odd o:
                    #   out_e = e*c - o*s ; out_o = e*s + o*c
                    # t1 stays in PSUM: the sub/add read t1 and t2 at
                    # different base partitions, which the ISA only allows
                    # when one operand is in PSUM.
                    t1 = psg.tile([P, CH], dt.float32, tag="t1", bufs=2)
                    t2 = t2p.tile([P, CH], dt.float16, tag="t2")
                    nc.vector.tensor_mul(out=t1[:], in0=ps[:],
                                         in1=ct_sb[:, nsl])
                    nc.vector.tensor_mul(out=t2[:], in0=ps[:],
                                         in1=st_sb[:, nsl])
                    o_sl = outT[:, h, nsl]
                    nc.vector.tensor_sub(out=o_sl[0:64, :], in0=t1[0:64, :],
                                         in1=t2[64:128, :])
                    nc.vector.tensor_add(out=o_sl[64:128, :],
                                         in0=t2[0:64, :],
                                         in1=t1[64:128, :])

                # k chunk 0: quarters g0/g1 dc-outer across all heads so the
                # first matmul gates on wk_q0 + xc0_q0 only (~1MB); quarters
                # g2/g3 swept per-head so the 4 PSUM groups finish staggered
                # and their RoPEs interleave into the matmul stream instead
                # of dumping 16 serial DVE ops at once.
                ps0 = [psg.tile([P, CH], dt.float32, tag="gemm", bufs=6,
                                name=f"ps0_{h}") for h in range(H_LOC)]
                for g in range(NQ // 2):
                    for d4 in range(DQ):
                        dc = g * DQ + d4
                        for h in range(H_LOC):
                            nc.tensor.matmul(
                                ps0[h][:],
                                wk_q[g][:, d4, h * HD:(h + 1) * HD],
                                xc0_q[g][:, d4, :],
                                start=(dc == 0), stop=False)
                # warmup: load the scalar-engine exp table now so the first
                # real EXP in phase 2 doesn't pay ACT_TABLE_LOAD
                warm = t2p.tile([P, 8], dt.float16, tag="warm")
                nc.scalar.activation(warm[:], ct_sb[:, 0:8],
                                     mybir.ActivationFunctionType.Exp)
                for h in range(H_LOC):
                    for g in range(NQ // 2, NQ):
                        for d4 in range(DQ):
                            dc = g * DQ + d4
                            nc.tensor.matmul(
                                ps0[h][:],
                                wk_q[g][:, d4, h * HD:(h + 1) * HD],
                                xc0_q[g][:, d4, :],
                                start=False, stop=(dc == DCH - 1))
                    rope(ps0[h], h, slice(0, CH), kTp)

                def proj_rope(w_q, outT, nchunk, xc=None):
                    """One n-chunk of a q/k projection + RoPE into outT."""
                    nsl = slice(nchunk * CH, (nchunk + 1) * CH)
                    if xc is None:
                        xc = xcp.tile([P, DCH, CH], dt.float16, tag="xc")
                        nc.sync.dma_start(xc[:], xTc[nchunk])
                    for h in range(H_LOC):
                        ps = psg.tile([P, CH], dt.float32, tag="gemm", bufs=6)
                        for dc in range(DCH):
                            nc.tensor.matmul(
                                ps[:],
                                w_q[dc // DQ][:, dc % DQ, h * HD:(h + 1) * HD],
                                xc[:, dc, :],
                                start=(dc == 0), stop=(dc == DCH - 1))
                        rope(ps, h, nsl, outT)

                for nchunk in range(1, NCH):
                    proj_rope(wk_q, kTp, nchunk, xc=xc1 if nchunk == 1 else None)
                nc.sync.dma_start(wv_sb[:], wvT[:])
                nc.sync.dma_start(vxc0[:], xTc[0])
                nc.sync.dma_start(ones_sb[:], ones[:])
                for nchunk in range(NCH):
                    proj_rope(wq_q, qTp, nchunk)
                # head of the v projection, emitted while the phase-1 PSUM
                # pool is still open: covers the pool-reuse barrier that
                # otherwise stalls the PE on the last rope's PSUM reads
                for nt in range(2):
                    ps = psg.tile([P, CH], dt.float32, tag="gemm", bufs=6,
                                  name=f"vhead_{nt}")
                    for dc in range(DCH):
                        nc.tensor.matmul(
                            ps[:], vxc0[:, dc, nt * P:(nt + 1) * P],
                            wv_sb[:, dc, :],
                            start=(dc == 0), stop=(dc == DCH - 1))
                    nc.vector.tensor_copy(v_sb[:, nt, :], ps[:])

            # ---- phase 2: scores pipeline + v + pv + projection ----------
            with (
                tc.tile_pool(name="etp", bufs=24) as etp,
                tc.tile_pool(name="attnp", bufs=2) as attnp,
                tc.tile_pool(name="accp", bufs=2) as accp,
                tc.tile_pool(name="ytp", bufs=3) as ytp,
                tc.tile_pool(name="psc", bufs=1, space="PSUM") as psc,
            ):
                acc_of = {}
                from collections import deque
                sc_iters = deque()
                appended = set()

                def appendgen(i):
                    if i < NB and i not in appended:
                        appended.add(i)
                        sc_iters.append(scores_gen(i))

                def scores_gen(b):
                    """Emit one score+exp+acc unit (2 matmuls) per yield, so
                    callers can interleave units with other TensorE work."""
                    qc, h = divmod(b, H_LOC)
                    qsl = slice(qc * CH, (qc + 1) * CH)
                    ets = []
                    acc = accp.tile([P, CH], dt.float16, tag="acc")
                    acc_of[b] = (acc, ets)
                    for ktp in range(NT // 2):
                        ss = psc.tile([P, 2, CH], dt.float32, tag="ss", bufs=2)
                        for i in range(2):
                            kt = 2 * ktp + i
                            nc.tensor.matmul(
                                ss[:, i, :], kTp[:, h, kt * P:(kt + 1) * P],
                                qTp[:, h, qsl], start=True, stop=True)
                        et = etp.tile([P, 2, CH], dt.float16, tag="et")
                        nc.scalar.activation(
                            et[:], ss[:], mybir.ActivationFunctionType.Exp)
                        if ktp == 0:
                            nc.vector.tensor_add(out=acc[:], in0=et[:, 0, :],
                                                 in1=et[:, 1, :])
                        else:
                            tmp = accp.tile([P, CH], dt.float16, tag="tmp",
                                            bufs=2)
                            nc.vector.tensor_add(out=tmp[:], in0=et[:, 0, :],
                                                 in1=et[:, 1, :])
                            nc.vector.tensor_add(out=acc[:], in0=acc[:],
                                                 in1=tmp[:])
                        ets.append(et)
                        yield

                def pump(n=1):
                    for _ in range(n):
                        while sc_iters:
                            try:
                                next(sc_iters[0])
                                break
                            except StopIteration:
                                sc_iters.popleft()

                rec_of = {}

                def emit_denom(b):
                    # softmax denominator = per-partition row sum of acc,
                    # done as a ones-matmul on the PE (432ns) instead of a
                    # 3.5us GpSimd partition_all_reduce; the reciprocal is
                    # emitted immediately so the PSUM slot frees fast and
                    # the attn multiply only ever waits on SBUF data.
                    acc, _ = acc_of[b]
                    dn = psc.tile([P, CH], dt.float32, tag="py", bufs=2,
                                  name=f"dn_{b}")
                    nc.tensor.matmul(dn[:], ones_sb[:], acc[:],
                                     start=True, stop=True)
                    rec = accp.tile([P, CH], dt.float32, tag="rec", bufs=2,
                                    name=f"rec_{b}")
                    nc.vector.reciprocal_approx_fast(rec[:], dn[:])
                    rec_of[b] = rec

                def pv_block(b, attn_cur):
                    qc, h = divmod(b, H_LOC)
                    hsl = slice(h * HD, (h + 1) * HD)
                    acc, ets = acc_of.pop(b)
                    pv = psc.tile([P, CH], dt.float32, tag="pv", bufs=2)
                    for ktp in range(NT // 2):
                        et = ets[ktp]
                        for i in range(2):
                            kt = 2 * ktp + i
                            nc.tensor.matmul(
                                pv[:], v_sb[:, kt, hsl], et[:, i, :],
                                start=(kt == 0), stop=(kt == NT - 1))
                        pump(1)
                        if ktp == 5 and b + 1 < NB:
                            emit_denom(b + 1)
                    rec = rec_of.pop(b)
                    nc.vector.tensor_mul(
                        out=attn_cur[:, h, :], in0=pv[:], in1=rec[:])

                def proj_chunk(qc, attn_cur, ntls=range(NCH), fine=False):
                    for ntl in ntls:
                        nt = qc * NCH + ntl
                        last = fine and ntl == NCH - 1
                        for half in range(2):
                            yt = ytp.tile([P, D // 2], dt.float16, tag="yt")
                            for i in range(2):
                                oc = half * 2 + i
                                py = psc.tile([P, CH], dt.float32, tag="py",
                                              bufs=2)
                                for h in range(H_LOC):
                                    nc.tensor.matmul(
                                        py[:],
                                        attn_cur[:, h, ntl * P:(ntl + 1) * P],
                                        wo_sb[:, h, oc * CH:(oc + 1) * CH],
                                        start=(h == 0), stop=(h == H_LOC - 1))
                                # PSUM eviction on DVE: ScalarE is the EXP
                                # engine and is near-saturated; these copies
                                # head-of-line-blocked the next block's exps
                                nc.vector.tensor_copy(
                                    yt[:, i * CH:(i + 1) * CH], py[:])
                                if last:
                                    # final row tile: per-oc DMAs so only a
                                    # quarter-size transfer drains after the
                                    # last matmul
                                    nc.sync.dma_start(
                                        y3[:, nt, oc * CH:(oc + 1) * CH],
                                        yt[:, i * CH:(i + 1) * CH])
                            if not last:
                                nc.sync.dma_start(
                                    y3[:, nt,
                                       half * D // 2:(half + 1) * D // 2],
                                    yt[:])
                            pump(1)

                # v projection, with the first two score blocks pumped
                # in fine-grained units between v PSUM groups
                appendgen(0)
                appendgen(1)
                for nchunk in range(NCH):
                    nsl = slice(nchunk * CH, (nchunk + 1) * CH)
                    if nchunk == 0:
                        xc = vxc0
                    else:
                        xc = xcp.tile([P, DCH, CH], dt.float16, tag="xc")
                        nc.sync.dma_start(xc[:], xTc[nchunk])
                    for nt in range(NCH):
                        if nchunk == 0 and nt < 2:
                            continue          # emitted in phase 1
                        ps = psc.tile([P, CH], dt.float32, tag="pv", bufs=2)
                        for dc in range(DCH):
                            nc.tensor.matmul(
                                ps[:], xc[:, dc, nt * P:(nt + 1) * P],
                                wv_sb[:, dc, :],
                                start=(dc == 0), stop=(dc == DCH - 1))
                        nc.vector.tensor_copy(
                            v_sb[:, nchunk * NCH + nt, :], ps[:])
                        pump(2 if nchunk == 0 else 1)
                    if nchunk == 0:
                        nc.sync.dma_start(wo_sb[:], woT[:])

                # steady state: [pv(b) | scores(b+2) units | proj(qc-1)]
                NB = NCH * H_LOC
                attn_hist = {}
                emit_denom(0)
                for b in range(NB):
                    qc = b // H_LOC
                    if b % H_LOC == 0:
                        attn_hist[qc] = attnp.tile([P, H_LOC, CH], dt.float16,
                                                   tag="attn",
                                                   name=f"attn_{qc}")
                    appendgen(b + 2)
                    if b % H_LOC == 0:
                        # stage the +3 block too so the proj chunk's pumps
                        # can run its exp/acc chain on the otherwise-idle
                        # ScalarE during the projection
                        appendgen(b + 3)
                    pv_block(b, attn_hist[qc])
                    if b % H_LOC == 0 and b > 0:
                        proj_chunk(qc - 1, attn_hist.pop(qc - 1))
                pump(100)
                proj_chunk(NCH - 1, attn_hist.pop(NCH - 1), fine=True)

    nc.compile()
    return nc


_NC_CACHE = None


def _get_program():
    global _NC_CACHE
    if _NC_CACHE is None:
        _NC_CACHE = _build_program()
    return _NC_CACHE


def _rope_tables():
    scale = np.arange(0, HD, 2, dtype=np.float32) / HD
    inv_freq = 1.0 / (10000.0 ** scale)                 # [64]
    t = np.arange(S, dtype=np.float32)
    ang = np.outer(t, inv_freq)                         # [S, 64]
    cos = np.cos(ang).T.astype(np.float32)              # [64, S]
    sin = np.sin(ang).T.astype(np.float32)
    stk = lambda a: np.ascontiguousarray(
        np.concatenate([a, a], axis=0)).astype(_F16)    # [128, S]
    return stk(cos), stk(sin)


def prepare_in_maps(x, wq, wk, wv, wo):
    x = np.asarray(x, dtype=np.float32)
    wq = np.asarray(wq, dtype=np.float32) * np.float32(1.0 / np.sqrt(HD))
    wk = np.asarray(wk, dtype=np.float32)
    wv = np.asarray(wv, dtype=np.float32)
    wo = np.asarray(wo, dtype=np.float32)

    ct_t, st_t = _rope_tables()

    # even/odd RoPE permutation of rows within each head
    perm = np.concatenate([np.arange(0, HD, 2), np.arange(1, HD, 2)])

    # [NCH, P, DCH, CH]: per-partition-contiguous x chunks
    xTc = [np.ascontiguousarray(
        x[b].T.reshape(DCH, P, NCH, CH).transpose(2, 1, 0, 3)).astype(_F16)
        for b in range(B)]

    in_maps = []
    for c in range(N_CORES):
        b, hg = divmod(c, H_LOC)
        heads = np.arange(hg * H_LOC, (hg + 1) * H_LOC)
        rows_qk = (heads[:, None] * HD + perm[None, :]).reshape(-1)  # [512]
        rows_nat = np.arange(hg * F, (hg + 1) * F)
        def pmaj(wT, groups):  # [D_in, F] -> [P, groups, F]
            return np.ascontiguousarray(
                wT.reshape(groups, P, wT.shape[1]).transpose(1, 0, 2)
            ).astype(_F16)
        in_maps.append({
            "xTc": xTc[b],
            "wqT": pmaj(wq[rows_qk].T, DCH),
            "wkT": pmaj(wk[rows_qk].T, DCH),
            "wvT": pmaj(wv[rows_nat].T, DCH),
            "woT": pmaj(wo[:, rows_nat].T, H_LOC),
            "ct": ct_t, "st": st_t,
            "ones": np.ones((P, P), dtype=_F16),
        })
    return in_maps


def combine_results(results):
    out = np.zeros((B, S, D), dtype=np.float32)
    for c, r in enumerate(results):
        out[c // H_LOC] += np.asarray(r["y"], dtype=np.float32)
    return out


def kernel(x, wq, wk, wv, wo):
    from concourse.bass_utils import run_bass_kernel_spmd

    nc = _get_program()
    in_maps = prepare_in_maps(x, wq, wk, wv, wo)
    res = run_bass_kernel_spmd(nc, in_maps, core_ids=list(range(N_CORES)))
    return combine_results(res.results)


if __name__ == "__main__":
    rng = np.random.default_rng(0)
    ins = {
        "x": rng.standard_normal((B, S, D), dtype=np.float32),
        "wq": rng.standard_normal((D, D), dtype=np.float32) / np.sqrt(D),
        "wk": rng.standard_normal((D, D), dtype=np.float32) / np.sqrt(D),
        "wv": rng.standard_normal((D, D), dtype=np.float32) / np.sqrt(D),
        "wo": rng.standard_normal((D, D), dtype=np.float32) / np.sqrt(D),
    }
    out = kernel(**ins)
    print("out", out.shape, out.dtype, np.abs(out).max())
